# revision 20
# baseline (speedup 1.0000x reference)
"""Trainium2 Bass kernel for a dense transformer block (pre-LN, causal MHA + FFN).

Shapes (hardcoded): x [1024, 64, 384] fp32, 6 heads x 64, FFN hidden 1536.
Strategy: data-parallel over batch across 8 NeuronCores (128 seqs/core), no
collectives. Per core, one fused loop over segments of 8 token tiles
(16 sequences): LN1 -> QKV -> causal attention -> proj+residual -> LN2 ->
FFN+residual. LN/softmax/residual math in fp32.

Precision: the six weight GEMMs (q/k/v proj, output proj, FFN1, FFN2) run in
fp8e4 with DoubleRow perf mode (two 128-row contraction planes per matmul,
2 MACs/PE-cell/cycle). K=384 contractions emit one DR pair + one plain fp8
matmul; FFN2's K=1536 is 6 pure DR pairs. Weights are scaled x64 host-side so
~0.02-magnitude entries clear the fp8e4 subnormal floor; the 1/64 (or 1/4096
for FFN2's doubly-scaled psum) is folded into the PSUM-evacuating activation
scale or a fused scalar_tensor_tensor (psum*s + residual). The attention
score/AV path (q,k,v values, exp, mask, AV) stays bf16/fp32.

Layout notes (contraction must sit on SBUF partitions for both operands):
 - xnF: LN1 output transposed to feature-major via PE transposes; serves as
   moving operand for q/k projections and stationary operand for v.
 - attention computes S^T = k @ q^T directly (scores transposed, [s, t]) so
   the softmax matrix is already stationary-ready for the AV matmul; the
   softmax denominator comes free as an extra ones-column in the v operand.
 - softmax skips the max-subtraction: scores are O(1) by construction
   (LN'd activations times 0.02-scale weights), exp is safe in fp32.
"""

import os
import sys

import numpy as np

for _p in ("/opt/trn_rl_repo", os.path.expanduser("~/.axon_site/_ro/trn_rl_repo")):
    if os.path.isdir(_p) and _p not in sys.path:
        sys.path.insert(0, _p)

import ml_dtypes  # noqa: E402
import concourse.hw_specs as _hw_specs  # noqa: E402
import concourse.bacc as bacc  # noqa: E402
import concourse.tile as tile  # noqa: E402
from concourse import mybir  # noqa: E402
from concourse.bass_utils import run_bass_kernel_spmd  # noqa: E402

# Pin every activation function this kernel uses (Exp/Ln/Identity/Copy/Relu)
# to the one act table that contains them all (natural_log_exp_and_others,
# set id 6). The default per-instruction table choice ping-pongs between
# tables, costing a ~1.3us table reload per switch on the ACT engine. Table
# ids keep their true act_info.json positions, so walrus emits correct
# act.json entries.
_ACT_PIN = {mybir.ActivationFunctionType.Exp, mybir.ActivationFunctionType.Ln,
            mybir.ActivationFunctionType.Identity,
            mybir.ActivationFunctionType.Copy,
            mybir.ActivationFunctionType.Relu}
_orig_get_tables = _hw_specs.get_activation_tables


def _pinned_tables(arch):
    out = {}
    for name, fns in _orig_get_tables(arch).items():
        out[name] = fns if name == "natural_log_exp_and_others" \
            else fns - _ACT_PIN
    return out


_hw_specs.get_activation_tables = _pinned_tables
bacc.get_activation_tables = _pinned_tables

BF16 = mybir.dt.bfloat16
F32 = mybir.dt.float32
FP8 = mybir.dt.float8e4
DRM = mybir.MatmulPerfMode.DoubleRow
ACTF = mybir.ActivationFunctionType
ALU = mybir.AluOpType

N_CORES = 8
B_FULL, T, C, H, D = 1024, 64, 384, 6, 64
J = 4 * C                       # 1536
B_LOC = B_FULL // N_CORES       # 128 sequences per core
NTOK = B_LOC * T                # 8192 tokens per core
P = 128
NT = NTOK // P                  # 64 token tiles (each tile = one pair of seqs)
SEG = 8                         # token tiles per fused segment
KC = C // P                     # 3 contraction chunks over C
JC = J // P                     # 12 chunks over FFN hidden
EPS = 1e-5
SCALE = D ** -0.5
WS = 64.0                       # fp8 weight scale (clears e4m3 subnormals)
SI = 1.0 / WS                   # descale for singly-scaled psums
SI2 = 1.0 / (WS * WS)           # descale for FFN2 (h64 @ w2x64)

_CACHE = {}
last_exec_time_ns = None


def _build(has_bv, has_bo, has_b2, nt=NT, loop_n=1):
    assert nt % SEG == 0 and (SEG * P) % 512 == 0
    nc = bacc.Bacc("TRN2", target_bir_lowering=False, debug=False)
    ntok = nt * P
    nseg = nt // SEG
    SW = SEG * P                # tokens per segment (1024)

    x_d = nc.dram_tensor("x", [ntok, C], F32, kind="ExternalInput").ap()
    wq_d = nc.dram_tensor("wq", [P, 2 * C], FP8, kind="ExternalInput").ap()
    wq2_d = nc.dram_tensor("wq2", [P, C], FP8, kind="ExternalInput").ap()
    wk_d = nc.dram_tensor("wk", [P, 2 * C], FP8, kind="ExternalInput").ap()
    wk2_d = nc.dram_tensor("wk2", [P, C], FP8, kind="ExternalInput").ap()
    wv_d = nc.dram_tensor("wv", [P, 2 * C], FP8, kind="ExternalInput").ap()
    wv2_d = nc.dram_tensor("wv2", [P, C], FP8, kind="ExternalInput").ap()
    wo_d = nc.dram_tensor("wo", [P, 2 * C], FP8, kind="ExternalInput").ap()
    wo2_d = nc.dram_tensor("wo2", [P, C], FP8, kind="ExternalInput").ap()
    w1_d = nc.dram_tensor("w1", [P, 2 * J], FP8, kind="ExternalInput").ap()
    w12_d = nc.dram_tensor("w12", [P, J], FP8, kind="ExternalInput").ap()
    w2_d = nc.dram_tensor("w2", [P, JC * C], FP8, kind="ExternalInput").ap()
    bq_d = nc.dram_tensor("bq", [P, KC], F32, kind="ExternalInput").ap()
    bk_d = nc.dram_tensor("bk", [P, KC], F32, kind="ExternalInput").ap()
    bh_d = nc.dram_tensor("bh", [P, JC], F32, kind="ExternalInput").ap()
    bv_d = nc.dram_tensor("bv", [1, C], BF16, kind="ExternalInput").ap()
    bo_d = nc.dram_tensor("bo_r", [1, C], BF16, kind="ExternalInput").ap()
    b2_d = nc.dram_tensor("b2_r", [1, C], BF16, kind="ExternalInput").ap()
    id_d = nc.dram_tensor("ident", [P, P], BF16, kind="ExternalInput").ap()
    mk_d = nc.dram_tensor("maskt", [P, H * P], BF16, kind="ExternalInput").ap()
    out_d = nc.dram_tensor("out", [ntok, C], F32, kind="ExternalOutput").ap()

    with tile.TileContext(nc) as tc:
        with tc.tile_pool(name="singles", bufs=1) as sg, \
             tc.tile_pool(name="seg", bufs=2) as sgp, \
             tc.tile_pool(name="work", bufs=5) as wk, \
             tc.tile_pool(name="psum", bufs=1, space="PSUM") as ps:

            # ---- resident weights / constants (fp8, x64-scaled) ----
            wqp = sg.tile([P, 2, C], FP8, name="wqp")
            wq2 = sg.tile([P, C], FP8, name="wq2")
            wkp = sg.tile([P, 2, C], FP8, name="wkp")
            wk2 = sg.tile([P, C], FP8, name="wk2")
            wvp = sg.tile([P, 2, C], FP8, name="wvp")
            wv2 = sg.tile([P, C], FP8, name="wv2")
            wop = sg.tile([P, 2, C], FP8, name="wop")
            wo2 = sg.tile([P, C], FP8, name="wo2")
            w1p = sg.tile([P, 2, J], FP8, name="w1p")
            w12 = sg.tile([P, J], FP8, name="w12")
            w2p = sg.tile([P, JC, C], FP8, name="w2p")
            nc.gpsimd.dma_start(out=wqp,
                                in_=wq_d.rearrange("p (k c) -> p k c", k=2))
            nc.gpsimd.dma_start(out=wq2, in_=wq2_d)
            nc.gpsimd.dma_start(out=wkp,
                                in_=wk_d.rearrange("p (k c) -> p k c", k=2))
            nc.gpsimd.dma_start(out=wk2, in_=wk2_d)
            nc.gpsimd.dma_start(out=wvp,
                                in_=wv_d.rearrange("p (k c) -> p k c", k=2))
            nc.gpsimd.dma_start(out=wv2, in_=wv2_d)
            nc.gpsimd.dma_start(out=wop,
                                in_=wo_d.rearrange("p (k c) -> p k c", k=2))
            nc.gpsimd.dma_start(out=wo2, in_=wo2_d)
            nc.gpsimd.dma_start(out=w1p,
                                in_=w1_d.rearrange("p (k c) -> p k c", k=2))
            nc.gpsimd.dma_start(out=w12, in_=w12_d)
            nc.gpsimd.dma_start(out=w2p,
                                in_=w2_d.rearrange("p (j c) -> p j c", j=JC))
            bq_sb = sg.tile([P, KC], F32)
            bk_sb = sg.tile([P, KC], F32)
            bh_sb = sg.tile([P, JC], F32)
            ident = sg.tile([P, P], BF16)
            maskt = sg.tile([P, H * P], BF16)
            nc.sync.dma_start(out=ident, in_=id_d)
            nc.scalar.dma_start(out=bq_sb, in_=bq_d)
            nc.scalar.dma_start(out=bk_sb, in_=bk_d)
            nc.scalar.dma_start(out=bh_sb, in_=bh_d)
            nc.scalar.dma_start(out=maskt, in_=mk_d)
            eps_sb = sg.tile([P, 1], F32)
            nc.vector.memset(eps_sb, EPS)
            ones1 = sg.tile([1, P], BF16)
            nc.vector.memset(ones1, 1.0)
            bv_sb = sg.tile([1, C], BF16)
            bo_sb = sg.tile([1, C], BF16)
            b2_sb = sg.tile([1, C], BF16)
            if has_bv:
                nc.sync.dma_start(out=bv_sb, in_=bv_d)
            if has_bo:
                nc.sync.dma_start(out=bo_sb, in_=bo_d)
            if has_b2:
                nc.sync.dma_start(out=b2_sb, in_=b2_d)

            def bass_strided(dstF, t):
                # [P, KC, 128] view of dstF hitting columns k*SW + t*128
                return dstF.rearrange("p (k w) -> p k w", k=KC)[
                    :, :, t * P:(t + 1) * P]

            def _copy(idx, out, in_):
                if idx % 2 == 0:
                    nc.scalar.copy(out=out, in_=in_)
                else:
                    nc.vector.tensor_copy(out=out, in_=in_)

            def ln_stats(src_f32, mvs, t):
                """bn stats of a [128, C] fp32 tile -> mvs[:, t, :] = (mu,
                var). The expensive per-column scalar chain (ln/exp/mul) is
                batched over half-segments in ln_batch."""
                stats = wk.tile([P, 6], F32, tag="lnstats")
                nc.vector.bn_stats(out=stats, in_=src_f32)
                nc.vector.bn_aggr(out=mvs[:, t, :], in_=stats)

            def ln_batch(mvs, rsts, nmrs, t0, t1):
                """rstd = exp(-0.5*ln(var+eps)) for tiles t0..t1 in one op
                per stage (keeps every ACT op in the one
                natural_log_exp_and_others table; amortizes op overhead)."""
                n = t1 - t0
                lnv = wk.tile([P, SEG], F32, tag="lnlnv")
                nc.scalar.activation(out=lnv[:, 0:n], in_=mvs[:, t0:t1, 1],
                                     func=ACTF.Ln, bias=eps_sb, scale=1.0)
                nc.scalar.activation(out=rsts[:, t0:t1], in_=lnv[:, 0:n],
                                     func=ACTF.Exp, bias=0.0, scale=-0.5)
                nc.vector.scalar_tensor_tensor(
                    out=nmrs[:, t0:t1], in0=mvs[:, t0:t1, 0], scalar=-1.0,
                    in1=rsts[:, t0:t1], op0=ALU.mult, op1=ALU.mult)

            def ln_xn(src_f32, rsts, nmrs, t, tag):
                # (x * rstd) + (-mu*rstd), SBUF->SBUF: runs on GPSIMD (the
                # only engine with no PSUM port, so keep it fed with the
                # SBUF-only elementwise work).
                xn0 = wk.tile([P, C], BF16, tag=tag, bufs=4, name="xn0")
                nc.gpsimd.tensor_scalar(out=xn0, in0=src_f32,
                                        scalar1=rsts[:, t:t + 1],
                                        scalar2=nmrs[:, t:t + 1],
                                        op0=ALU.mult, op1=ALU.add)
                return xn0

            def tp_to_F(t, xn0, dstF, eng_off=0):
                tp = ps.tile([P, C], BF16, tag="big", bufs=2, name="tp")
                for k in range(KC):
                    nc.tensor.transpose(tp[:, k * P:(k + 1) * P],
                                        xn0[:, k * P:(k + 1) * P], ident)
                # one strided copy: chunk k lands at column k*SW + t*128
                _copy(t + eng_off, bass_strided(dstF, t),
                      tp.rearrange("p (k c) -> p k c", c=P))

            ng = SW // 512

            def new_state(s):
                st_ = {"i0": s * SEG}
                st_["xnF"] = sgp.tile([P, KC * SW], FP8, tag="xnF",
                                      name="xnF")
                st_["qF"] = [sgp.tile([P, SW], BF16, tag=f"qF{m}",
                                      name=f"qF{m}") for m in range(KC)]
                st_["kF"] = [sgp.tile([P, SW], BF16, tag=f"kF{m}",
                                      name=f"kF{m}") for m in range(KC)]
                st_["vaug"] = sgp.tile([P, SEG, H, D + 1], BF16, tag="vaug",
                                       name="vaug")
                st_["attn"] = sgp.tile([P, SEG * C], BF16, tag="attn",
                                       name="attn")
                st_["xn2F"] = sgp.tile([P, KC * SW], FP8, tag="xn2F",
                                       name="xn2F")
                st_["x2"] = sgp.tile([P, SEG, C], F32, tag="x2", name="x2")
                st_["mv1"] = sgp.tile([P, SEG, 2], F32, tag="mv1", name="mv1")
                st_["rst1"] = sgp.tile([P, SEG], F32, tag="rst1", name="rst1")
                st_["nmr1"] = sgp.tile([P, SEG], F32, tag="nmr1", name="nmr1")
                st_["mv2"] = sgp.tile([P, SEG, 2], F32, tag="mv2", name="mv2")
                st_["rst2"] = sgp.tile([P, SEG], F32, tag="rst2", name="rst2")
                st_["nmr2"] = sgp.tile([P, SEG], F32, tag="nmr2", name="nmr2")
                return st_

            def emit_A_ln(st_, t):
                xt = wk.tile([P, C], F32, tag="xa", bufs=6)
                i0 = st_["i0"]
                nc.sync.dma_start(
                    out=xt, in_=x_d[(i0 + t) * P:(i0 + t + 1) * P, :])
                ln_stats(xt, st_["mv1"], t)
                st_[f"xt{t}"] = xt

            def emit_A_half(st_, h):
                ln_batch(st_["mv1"], st_["rst1"], st_["nmr1"],
                         4 * h, 4 * h + 4)

            def emit_A_xtp(st_, t):
                xn0 = ln_xn(st_.pop(f"xt{t}"), st_["rst1"], st_["nmr1"], t,
                            "lnxn0A")
                tp_to_F(t, xn0, st_["xnF"])

            def emit_B(st_):
                xnF, qF, kF = st_["xnF"], st_["qF"], st_["kF"]
                x3 = xnF.rearrange("p (k w) -> p k w", k=KC)
                for m in range(KC):
                    for g in range(ng):
                        for iqk, (wp_, w2c, dstF, bias) in enumerate(
                                ((wqp, wq2, qF, bq_sb),
                                 (wkp, wk2, kF, bk_sb))):
                            pqk = ps.tile([P, 512], F32, tag="st", bufs=2)
                            nc.tensor.matmul(
                                pqk, wp_[:, :, m * P:(m + 1) * P],
                                x3[:, 0:2, g * 512:(g + 1) * 512],
                                start=True, stop=False, perf_mode=DRM)
                            nc.tensor.matmul(
                                pqk, w2c[:, m * P:(m + 1) * P],
                                xnF[:, 2 * SW + g * 512:
                                    2 * SW + (g + 1) * 512],
                                start=False, stop=True)
                            dst = dstF[m][:, g * 512:(g + 1) * 512]
                            if (2 * m + iqk + g) % 2 == 0:
                                nc.scalar.activation(
                                    out=dst, in_=pqk, func=ACTF.Identity,
                                    bias=bias[:, m:m + 1], scale=SI)
                            else:
                                nc.vector.tensor_scalar(
                                    out=dst, in0=pqk, scalar1=SI,
                                    scalar2=bias[:, m:m + 1],
                                    op0=ALU.mult, op1=ALU.add)
                # v projection (xnF stationary -> T-layout, plus ones column)
                vaug = st_["vaug"]
                nc.vector.memset(vaug[:, :, :, D:D + 1], 1.0)
                for t in range(SEG):
                    pvf = ps.tile([P, 512], F32, tag="vf", bufs=2)
                    pv = pvf[:, 0:C]
                    nc.tensor.matmul(
                        pv, x3[:, 0:2, t * P:(t + 1) * P], wvp,
                        start=True, stop=False, perf_mode=DRM)
                    nc.tensor.matmul(
                        pv, xnF[:, 2 * SW + t * P:2 * SW + (t + 1) * P],
                        wv2, start=False, stop=(not has_bv))
                    if has_bv:
                        nc.tensor.matmul(pv, ones1, bv_sb, start=False,
                                         stop=True)
                    pvr = pv.rearrange("p (h d) -> p h d", h=H)
                    if t % 2 == 0:
                        nc.scalar.activation(out=vaug[:, t, :, 0:D], in_=pvr,
                                             func=ACTF.Identity, bias=0.0,
                                             scale=SI)
                    else:
                        nc.vector.tensor_scalar_mul(vaug[:, t, :, 0:D],
                                                    pvr, SI)

            def emit_S1a(st_, t):
                qF, kF = st_["qF"], st_["kF"]
                # attention: S^T computed as full [128,128] blocks per
                # (head-parity, chunk): both sequences of the pair at once.
                # Cross-sequence quadrants are garbage that the block-diagonal
                # causal mask zeroes before AV. Two psum banks by head parity
                # (HW forbids mixed PE row-groups per bank partition range).
                # em columns: block (hp, ch) at (hp*KC + ch) * 128.
                em = wk.tile([P, H * P], BF16, tag="em", bufs=5)
                for hp in range(2):
                    sthf = ps.tile([P, 512], F32, tag="st", bufs=2,
                                   name="sth")
                    sth = sthf[:, 0:KC * P]
                    pb = hp * 64
                    for ch in range(KC):
                        nc.tensor.matmul(
                            sth[:, ch * P:(ch + 1) * P],
                            kF[ch][pb:pb + 64, t * P:(t + 1) * P],
                            qF[ch][pb:pb + 64, t * P:(t + 1) * P],
                            start=True, stop=True)
                    nc.scalar.activation(
                        out=em[:, hp * KC * P:(hp + 1) * KC * P], in_=sth,
                        func=ACTF.Exp, bias=0.0, scale=SCALE)
                nc.gpsimd.tensor_mul(out=em, in0=em, in1=maskt)
                st_[f"em{t}"] = em

            def emit_S1b(st_, t):
                vaug, attn = st_["vaug"], st_["attn"]
                em = st_.pop(f"em{t}")
                avf = ps.tile([P, 512], F32, tag="avpr", bufs=2)
                av = avf[:, 0:H * (D + 1)].rearrange("p (h e) -> p h e",
                                                     e=D + 1)
                for ch in range(KC):
                    for hp in range(2):
                        h = 2 * ch + hp
                        bc = (hp * KC + ch) * P
                        nc.tensor.matmul(
                            av[:, h, :], em[:, bc:bc + P],
                            vaug[:, t, h, :], start=True, stop=True)
                invl = wk.tile([P, H], F32, tag="invl")
                nc.vector.reciprocal(
                    out=invl, in_=av[:, :, D:D + 1].rearrange("p h 1 -> p h"))
                nc.vector.tensor_mul(
                    out=attn[:, t * C:(t + 1) * C].rearrange(
                        "p (h d) -> p h d", h=H),
                    in0=av[:, :, 0:D],
                    in1=invl.unsqueeze(2).broadcast_to([P, H, D]))

            def emit_D_tile(st_, t):
                i0, attn, x2 = st_["i0"], st_["attn"], st_["x2"]
                # proj + residual + LN2 -> xn2F
                aoF = wk.tile([P, KC * P], FP8, tag="aoF", bufs=3)
                tp = ps.tile([P, C], BF16, tag="big", bufs=2, name="tp")
                for k in range(KC):
                    nc.tensor.transpose(
                        tp[:, k * P:(k + 1) * P],
                        attn[:, t * C + k * P: t * C + (k + 1) * P], ident)
                _copy(t, aoF, tp)
                pprf = ps.tile([P, 512], F32, tag="avpr", bufs=2)
                ppr = pprf[:, 0:C]
                ao3 = aoF.rearrange("p (k c) -> p k c", k=KC)
                nc.tensor.matmul(ppr, ao3[:, 0:2, :], wop,
                                 start=True, stop=False, perf_mode=DRM)
                nc.tensor.matmul(ppr, aoF[:, 2 * P:3 * P], wo2,
                                 start=False, stop=(not has_bo))
                if has_bo:
                    nc.tensor.matmul(ppr, ones1, bo_sb, start=False, stop=True)
                xt = wk.tile([P, C], F32, tag="xd")
                nc.sync.dma_start(
                    out=xt, in_=x_d[(i0 + t) * P:(i0 + t + 1) * P, :])
                nc.vector.scalar_tensor_tensor(
                    out=x2[:, t, :], in0=ppr, scalar=SI, in1=xt,
                    op0=ALU.mult, op1=ALU.add)
                ln_stats(x2[:, t, :], st_["mv2"], t)

            def emit_D_half(st_, h):
                ln_batch(st_["mv2"], st_["rst2"], st_["nmr2"],
                         4 * h, 4 * h + 4)

            def emit_D3(st_, t):
                xn0d = ln_xn(st_["x2"][:, t, :], st_["rst2"], st_["nmr2"], t,
                             "lnxn0D")
                tp_to_F(t, xn0d, st_["xn2F"], eng_off=1)

            def emit_EF_chunk(st_, ph):
                # 8 phases per segment: per group g: E half-j, E half-j,
                # F tiles 0-1, F tiles 2-3
                g = ph // 4
                sub = ph % 4
                if sub == 0:
                    emit_E(st_, g, 0, JC // 2)
                elif sub == 1:
                    emit_E(st_, g, JC // 2, JC)
                elif sub == 2:
                    emit_F(st_, g, 0, 2)
                else:
                    emit_F(st_, g, 2, 4)

            def emit_E(st_, g, j0, j1):
                xn2F = st_["xn2F"]
                if f"hF{g}" not in st_:
                    st_[f"hF{g}"] = wk.tile([P, JC * 512], FP8,
                                            tag=f"hFg{g}", bufs=2,
                                            name=f"hFg{g}")
                hFg = st_[f"hF{g}"]
                xx3 = xn2F.rearrange("p (k w) -> p k w", k=KC)
                for j in range(j0, j1):
                    phf = ps.tile([P, 512], F32, tag="big", bufs=2)
                    nc.tensor.matmul(
                        phf, w1p[:, :, j * P:(j + 1) * P],
                        xx3[:, 0:2, g * 512:(g + 1) * 512],
                        start=True, stop=False, perf_mode=DRM)
                    nc.tensor.matmul(
                        phf, w12[:, j * P:(j + 1) * P],
                        xn2F[:, 2 * SW + g * 512:2 * SW + (g + 1) * 512],
                        start=False, stop=True)
                    hslice = hFg[:, j * 512:(j + 1) * 512]
                    if j % 2 == 0:
                        nc.scalar.activation(out=hslice, in_=phf,
                                             func=ACTF.Relu,
                                             bias=bh_sb[:, j:j + 1], scale=1.0)
                    else:
                        nc.vector.tensor_scalar(out=hslice, in0=phf,
                                                scalar1=bh_sb[:, j:j + 1],
                                                scalar2=0.0, op0=ALU.add,
                                                op1=ALU.max)

            def emit_F(st_, g, tg0, tg1):
                i0, x2 = st_["i0"], st_["x2"]
                hFg = st_[f"hF{g}"]
                h3 = hFg.rearrange("p (j w) -> p j w", j=JC)
                for tg in range(tg0, tg1):
                    t = g * (512 // P) + tg
                    pff = ps.tile([P, 512], F32, tag="vf", bufs=2)
                    pf = pff[:, 0:C]
                    for i in range(JC // 2):
                        nc.tensor.matmul(
                            pf, h3[:, 2 * i:2 * i + 2, tg * P:(tg + 1) * P],
                            w2p[:, 2 * i:2 * i + 2, :],
                            start=(i == 0),
                            stop=(i == JC // 2 - 1 and not has_b2),
                            perf_mode=DRM)
                    if has_b2:
                        nc.tensor.matmul(pf, ones1, b2_sb, start=False,
                                         stop=True)
                    ot = wk.tile([P, C], F32, tag="ot")
                    nc.vector.scalar_tensor_tensor(
                        out=ot, in0=pf, scalar=SI2, in1=x2[:, t, :],
                        op0=ALU.mult, op1=ALU.add)
                    nc.sync.dma_start(
                        out=out_d[(i0 + t) * P:(i0 + t + 1) * P, :], in_=ot)

            # ====== software-pipelined emission over segments ======
            # While segment s runs attention/proj (latency-bound, PE-sparse),
            # the instruction streams also carry segment s+1's LN1 loads and
            # segment s-1's FFN groups (PE-dense) to keep every engine fed.
            def _emit_all():
                cur = new_state(0)
                for t in range(SEG):
                    emit_A_ln(cur, t)
                    if t == 3:
                        emit_A_half(cur, 0)
                emit_A_half(cur, 1)
                for t in range(SEG):
                    emit_A_xtp(cur, t)
                prv = None
                NSL = SEG + 7
                for s in range(nseg):
                    emit_B(cur)
                    nxt = new_state(s + 1) if s + 1 < nseg else None
                    for t in range(NSL):
                        if t < SEG:
                            emit_S1a(cur, t)     # S^T matmuls + exp
                        if 2 <= t <= SEG + 1:
                            emit_S1b(cur, t - 2)  # mask, AV, normalize
                        if 3 <= t <= SEG + 2:
                            emit_D_tile(cur, t - 3)  # proj + resid + LN2 stats
                        if t == 6 or t == 10:
                            emit_D_half(cur, (t - 6) // 4)
                        if 7 <= t:
                            emit_D3(cur, t - 7)  # LN2 xn + transposes -> xn2F
                        if nxt is not None:
                            if t < SEG:
                                emit_A_ln(nxt, t)
                            if t == 3 or t == 7:
                                emit_A_half(nxt, (t - 3) // 4)
                            if 4 <= t <= SEG + 3:
                                emit_A_xtp(nxt, t - 4)
                        if prv is not None:
                            if t % 2 == 1 and t < 14:
                                emit_EF_chunk(prv, t // 2)
                            elif t == 14:
                                emit_EF_chunk(prv, 7)
                    prv, cur = cur, nxt
                for ph in range(8):
                    emit_EF_chunk(prv, ph)

            import contextlib
            loop_ctx = tc.For_i(0, loop_n) if loop_n > 1 \
                else contextlib.nullcontext()
            with loop_ctx:
                _emit_all()

    nc.compile()
    return nc


_FP8NP = mybir.dt.np(FP8)


def _bf16(a):
    return np.asarray(a, np.float32).astype(ml_dtypes.bfloat16)


def _fp8(a):
    a = np.asarray(a, np.float32)
    return np.clip(a, -240.0, 240.0).astype(_FP8NP)


def _dr_pack(w):
    """[C_in, N] -> (DR pair plane tile [P, 2*N], k2 tile [P, N])."""
    n = w.shape[1]
    pair = w[0:2 * P].reshape(2, P, n).transpose(1, 0, 2).reshape(P, 2 * n)
    return _fp8(pair), _fp8(w[2 * P:3 * P])


def _prep(ln1_g, ln1_b, Wq, Wk, Wv, Wo, bo, ln2_g, ln2_b, W1, b1, W2, b2):
    """Host-side weight prep: fold LN affine into weights, scale x64 to fp8,
    pack DoubleRow plane pairs, pack aux consts."""
    ln1_g = np.asarray(ln1_g, np.float32)
    ln1_b = np.asarray(ln1_b, np.float32)
    ln2_g = np.asarray(ln2_g, np.float32)
    ln2_b = np.asarray(ln2_b, np.float32)
    wq_all = np.asarray(Wq, np.float32).transpose(1, 0, 2).reshape(C, C)
    wk_all = np.asarray(Wk, np.float32).transpose(1, 0, 2).reshape(C, C)
    wv_all = np.asarray(Wv, np.float32).transpose(1, 0, 2).reshape(C, C)
    W1 = np.asarray(W1, np.float32)
    W2 = np.asarray(W2, np.float32)
    bq = ln1_b @ wq_all
    bk = ln1_b @ wk_all
    bv = ln1_b @ wv_all
    bh = np.asarray(b1, np.float32) + ln2_b @ W1
    causal_t = np.tril(np.ones((T, T), np.float32)).T  # [s, t]
    mask_bd = np.zeros((P, P), np.float32)  # block-diag causal^T for seq pair
    mask_bd[:T, :T] = causal_t
    mask_bd[T:, T:] = causal_t
    wq_s = WS * ln1_g[:, None] * wq_all
    wk_s = WS * ln1_g[:, None] * wk_all
    wv_s = WS * ln1_g[:, None] * wv_all
    wo_s = WS * np.asarray(Wo, np.float32)
    w1_s = WS * ln2_g[:, None] * W1
    w2_s = WS * W2
    wq_p, wq_2 = _dr_pack(wq_s)
    wk_p, wk_2 = _dr_pack(wk_s)
    wv_p, wv_2 = _dr_pack(wv_s)
    wo_p, wo_2 = _dr_pack(wo_s)
    w1_p, w1_2 = _dr_pack(w1_s)
    w2_p = _fp8(w2_s.reshape(JC, P, C).transpose(1, 0, 2).reshape(P, JC * C))
    d = {
        "wq": wq_p, "wq2": wq_2,
        "wk": wk_p, "wk2": wk_2,
        "wv": wv_p, "wv2": wv_2,
        "wo": wo_p, "wo2": wo_2,
        "w1": w1_p, "w12": w1_2,
        "w2": w2_p,
        "bq": bq.reshape(KC, P).T.copy(),
        "bk": bk.reshape(KC, P).T.copy(),
        "bh": (WS * bh).reshape(JC, P).T.copy(),
        "bv": _bf16(WS * bv).reshape(1, C),
        "bo_r": _bf16(WS * np.asarray(bo, np.float32)).reshape(1, C),
        "b2_r": _bf16(WS * WS * np.asarray(b2, np.float32)).reshape(1, C),
        "ident": np.eye(P, dtype=np.float32).astype(ml_dtypes.bfloat16),
        "maskt": _bf16(np.tile(mask_bd, (1, H))),
    }
    flags = (bool(np.any(bv != 0)), bool(np.any(np.asarray(bo) != 0)),
             bool(np.any(np.asarray(b2) != 0)))
    return d, flags


def kernel(x, ln1_g, ln1_b, Wq, Wk, Wv, Wo, bo, ln2_g, ln2_b, W1, b1, W2, b2):
    global last_exec_time_ns
    x = np.asarray(x, np.float32)
    aux, flags = _prep(ln1_g, ln1_b, Wq, Wk, Wv, Wo, bo, ln2_g, ln2_b, W1, b1,
                       W2, b2)
    key = flags
    if key not in _CACHE:
        _CACHE[key] = _build(*flags)
    nc = _CACHE[key]
    in_maps = []
    for c in range(N_CORES):
        m = dict(aux)
        m["x"] = x[c * B_LOC:(c + 1) * B_LOC].reshape(NTOK, C)
        in_maps.append(m)
    res = run_bass_kernel_spmd(nc, in_maps, list(range(N_CORES)))
    last_exec_time_ns = res.exec_time_ns
    out = np.stack([res.results[c]["out"] for c in range(N_CORES)])
    return out.reshape(B_FULL, T, C).astype(np.float32)
